# revision 2
# baseline (speedup 1.0000x reference)
"""Trainium2 Bass kernel for causal GQA multi-head attention (nn_MHA_79362405695575).

Full (unsharded) inputs -> full output. Internally: tensor-parallel over heads
across 8 NeuronCores. Core c owns q-heads [4c,4c+4) and kv-head c, computes its
partial out-projection, and chunked ReduceScatters sum partials; core c returns
a [256, 4096] shard of y^T (chunk-interleaved rows), which the host reassembles.

Reference semantics (fp32):
  q = x@Wq; k = x@Wk; v = x@Wv + bv           (B=2, S=2048, D=2048)
  q,k := interleaved RoPE(base 10000, hd=64)
  scores = q k^T / 8 (causal), attn = softmax
  out = attn @ v;  y = out @ Wo + bo

All matmuls run as float32r (TF32-class, ~2e-4 rel err, full PE rate).
Everything on-chip is transposed: qT/kT/vT [dim, row] layouts so no PE
transposes are needed anywhere in attention. Softmax is max-free (scores are
provably small) and denominators ride along the AV matmul as a 65th column
of v. Inputs arrive pre-tiled from the host so every DMA is a few large
contiguous transfers.
"""

import numpy as np

import concourse.bass as bass
import concourse.tile as tile
from concourse import bacc, mybir
from concourse.bass_utils import run_bass_kernel_spmd

# ---- problem constants (hardcoded; kernel.py must be self-contained) ----
B, S, D = 2, 2048, 2048
NH, NKV, HD = 32, 8, 64
ROPE_BASE = 10000.0
NC = 8                    # cores
HPC = NH // NC            # q heads per core = 4
R = B * S                 # 4096 rows
RS_N = 8                  # projection row spans
RS_W = R // RS_N          # 512 rows per span
QS_W = 512                # attention q-span width
QS_N = S // QS_W          # 4 q spans per batch
KB_W = 128                # k block width
NKB = S // KB_W           # 16 k blocks per batch
DCB = D // 128            # 16 out-proj column blocks
NCHK = 4                  # reduce-scatter chunks
CHW = D // NCHK           # 512 yT rows per chunk

F32 = mybir.dt.float32
F32R = mybir.dt.float32r

_CACHE = {}


def _build():
    nc = bacc.Bacc("TRN2", target_bir_lowering=False, debug=False, num_devices=NC)

    # ---- DRAM I/O (pre-tiled on host) ----
    xta = nc.dram_tensor("xta", [RS_N, 128, 8, RS_W], F32R, kind="ExternalInput").ap()
    xtb = nc.dram_tensor("xtb", [RS_N, 128, 8, RS_W], F32R, kind="ExternalInput").ap()
    wq = nc.dram_tensor("wq", [128, D // 128, 256], F32R, kind="ExternalInput").ap()
    wkv = nc.dram_tensor("wkv", [128, D // 128, 128], F32R, kind="ExternalInput").ap()
    wo = nc.dram_tensor("wo", [256, D], F32R, kind="ExternalInput").ap()
    bv_in = nc.dram_tensor("bv", [HD, 1], F32, kind="ExternalInput").ap()
    bo_in = nc.dram_tensor("bo", [128, 2], F32, kind="ExternalInput").ap()
    c4h = nc.dram_tensor("c4h", [128, S], F32, kind="ExternalInput").ap()
    s4h = nc.dram_tensor("s4h", [128, S], F32, kind="ExternalInput").ap()
    p2 = nc.dram_tensor("p2", [128, 128], F32R, kind="ExternalInput").ap()
    ident = nc.dram_tensor("ident", [64, 64], F32R, kind="ExternalInput").ap()
    masks = nc.dram_tensor("masks", [128, 4, HPC * QS_W], F32R, kind="ExternalInput").ap()
    ones32 = nc.dram_tensor("ones32", [128, R // KB_W], F32R, kind="ExternalInput").ap()
    y_sh = nc.dram_tensor("y_sh", [NCHK * HD, R], F32, kind="ExternalOutput").ap()

    DMA = nc.sync

    with tile.TileContext(nc) as tc:
        with (
            tc.tile_pool(name="persist", bufs=1) as pp,
            tc.tile_pool(name="dram", bufs=1, space="DRAM") as dram,
        ):
            # ---- persistent SBUF (whole kernel) ----
            qrT = [pp.tile([128, R], F32R, tag=f"qrT{t}", name=f"qrT{t}") for t in range(2)]
            krT = pp.tile([128, R], F32R, tag="krT")
            v_aug = pp.tile([128, R // KB_W, 65], F32R, tag="vaug")
            outT = [pp.tile([128, R], F32R, tag=f"outT{t}", name=f"outT{t}") for t in range(2)]
            p2_sb = pp.tile([128, 128], F32R, tag="p2")
            id_sb = pp.tile([64, 64], F32R, tag="ident")
            bv_sb = pp.tile([HD, 1], F32, tag="bv")
            bo_sb = pp.tile([128, 2], F32, tag="bo")

            DMA.dma_start(out=p2_sb[:], in_=p2[:])
            DMA.dma_start(out=id_sb[:], in_=ident[:])
            DMA.dma_start(out=bv_sb[:], in_=bv_in[:])
            DMA.dma_start(out=bo_sb[:], in_=bo_in[:])
            DMA.dma_start(out=v_aug[:, :, 64:65],
                          in_=ones32.rearrange("p (j o) -> p j o", o=1))

            yT_part = dram.tile([D, R], F32)
            rs_out = dram.tile([256, R], F32)

            # ================= stage 1: projections + RoPE =================
            with (
                tc.tile_pool(name="w1p", bufs=1) as w1p,
                tc.tile_pool(name="xtpa", bufs=2) as xtpa,
                tc.tile_pool(name="xtpb", bufs=1) as xtpb,
                tc.tile_pool(name="ropet", bufs=2) as ropet,
                tc.tile_pool(name="vstg", bufs=2) as vstg,
                tc.tile_pool(name="ps_q", bufs=2, space="PSUM") as ps_q,
                tc.tile_pool(name="ps_kv", bufs=2, space="PSUM") as ps_kv,
                tc.tile_pool(name="ps_sw", bufs=2, space="PSUM") as ps_sw,
                tc.tile_pool(name="ps_vt", bufs=1, space="PSUM") as ps_vt,
            ):
                wq_sb = w1p.tile([128, D // 128, 256], F32R, tag="wq")
                wkv_sb = w1p.tile([128, D // 128, 128], F32R, tag="wkv")
                c4_sb = w1p.tile([128, S], F32, tag="c4")
                s4_sb = w1p.tile([128, S], F32, tag="s4")
                DMA.dma_start(out=wq_sb[:], in_=wq[:])
                DMA.dma_start(out=wkv_sb[:], in_=wkv[:])
                DMA.dma_start(out=c4_sb[:], in_=c4h[:])
                DMA.dma_start(out=s4_sb[:], in_=s4h[:])
                SPB = RS_N // B          # spans per batch
                for rs in range(RS_N):
                    rsl = slice(rs * RS_W, (rs + 1) * RS_W)
                    ssl = slice((rs % SPB) * RS_W, (rs % SPB + 1) * RS_W)
                    xa = xtpa.tile([128, 8, RS_W], F32R, tag="xa")
                    xb = xtpb.tile([128, 8, RS_W], F32R, tag="xb")
                    DMA.dma_start(out=xa[:], in_=xta[rs])
                    DMA.dma_start(out=xb[:], in_=xtb[rs])

                    def xt(kb):
                        return xa[:, kb, :] if kb < 8 else xb[:, kb - 8, :]

                    # -- q projection: 2 colblocks (2 heads each) --
                    for cb in range(2):
                        pq = ps_q.tile([128, RS_W], F32, tag="pq")
                        for kb in range(D // 128):
                            nc.tensor.matmul(pq[:], wq_sb[:, kb, cb * 128:(cb + 1) * 128],
                                             xt(kb),
                                             start=(kb == 0), stop=(kb == D // 128 - 1))
                        # RoPE: qr = pq*C + P2.T @ (pq*S)
                        st = ropet.tile([128, RS_W], F32R, tag="st")
                        nc.vector.tensor_tensor(out=st[:], in0=pq[:], in1=s4_sb[:, ssl],
                                                op=mybir.AluOpType.mult)
                        sw = ps_sw.tile([128, RS_W], F32, tag="sw")
                        nc.tensor.matmul(sw[:], p2_sb[:], st[:], start=True, stop=True)
                        ct = ropet.tile([128, RS_W], F32, tag="ct")
                        nc.vector.tensor_tensor(out=ct[:], in0=pq[:], in1=c4_sb[:, ssl],
                                                op=mybir.AluOpType.mult)
                        nc.vector.tensor_tensor(out=qrT[cb][:, rsl], in0=ct[:], in1=sw[:],
                                                op=mybir.AluOpType.add)

                    # -- kv projection: cols 0:64 = kT(perm), 64:128 = vT --
                    pkv = ps_kv.tile([128, RS_W], F32, tag="pkv")
                    for kb in range(D // 128):
                        nc.tensor.matmul(pkv[:], wkv_sb[:, kb, :], xt(kb),
                                         start=(kb == 0), stop=(kb == D // 128 - 1))
                    # k RoPE (partitions 0:64), duplicated into krT[0:64] and [64:128]
                    stk = ropet.tile([64, RS_W], F32R, tag="stk")
                    nc.vector.tensor_tensor(out=stk[:], in0=pkv[0:64, :],
                                            in1=s4_sb[0:64, ssl], op=mybir.AluOpType.mult)
                    swk = ps_sw.tile([64, RS_W], F32, tag="sw")
                    nc.tensor.matmul(swk[:], p2_sb[0:64, 0:64], stk[:], start=True, stop=True)
                    ctk = ropet.tile([64, RS_W], F32, tag="ctk")
                    nc.vector.tensor_tensor(out=ctk[:], in0=pkv[0:64, :],
                                            in1=c4_sb[0:64, ssl], op=mybir.AluOpType.mult)
                    nc.vector.tensor_tensor(out=krT[0:64, rsl], in0=ctk[:], in1=swk[:],
                                            op=mybir.AluOpType.add)
                    nc.vector.tensor_tensor(out=krT[64:128, rsl], in0=ctk[:], in1=swk[:],
                                            op=mybir.AluOpType.add)

                    # v: bias add then transpose [64,128] -> [128,64] blocks
                    vst = vstg.tile([64, RS_W], F32R, tag="vst")
                    nc.scalar.activation(out=vst[:], in_=pkv[64:128, :],
                                         func=mybir.ActivationFunctionType.Identity,
                                         bias=bv_sb[:], scale=1.0)
                    for j in range(RS_W // KB_W):
                        pv = ps_vt.tile([128, 64], F32R, tag="pv")
                        nc.tensor.transpose(pv[:], vst[:, j * 128:(j + 1) * 128], id_sb[:])
                        nc.vector.tensor_copy(
                            out=v_aug[:, rs * (RS_W // KB_W) + j, 0:64], in_=pv[:])

            # ============ stage 2 + 3: attention, out-proj, chunked RS ============
            with tc.tile_pool(name="w2p", bufs=1) as w2p:
                wo_sb = [w2p.tile([128, D], F32R, tag=f"wo{t}", name=f"wo{t}")
                         for t in range(2)]
                mask_sb = w2p.tile([128, 4, HPC * QS_W], F32R, tag="masks")
                DMA.dma_start(out=wo_sb[0][:], in_=wo[0:128, :])
                DMA.dma_start(out=wo_sb[1][:], in_=wo[128:256, :])
                DMA.dma_start(out=mask_sb[:], in_=masks[:])

                with (
                    tc.tile_pool(name="ptp", bufs=3) as ptp,
                    tc.tile_pool(name="normp", bufs=2) as normp,
                    tc.tile_pool(name="ps_s", bufs=2, space="PSUM") as ps_s,
                    tc.tile_pool(name="ps_av", bufs=1, space="PSUM") as ps_av,
                ):
                    for b in range(B):
                        for qs in range(QS_N):
                            n_kb = 4 * (qs + 1)
                            qsl = slice(b * S + qs * QS_W, b * S + (qs + 1) * QS_W)
                            pav = ps_av.tile([65, HPC * QS_W], F32, tag="pav")
                            for kb in range(n_kb):
                                kbl = slice(b * S + kb * KB_W, b * S + (kb + 1) * KB_W)
                                dlt = kb - 4 * qs
                                for g in range(2):
                                    pss = ps_s.tile([128, 2 * QS_W], F32, tag="pss")
                                    nc.tensor.matmul(
                                        pss[:, 0:QS_W],
                                        krT[0:64, kbl], qrT[g][0:64, qsl],
                                        start=True, stop=True)
                                    nc.tensor.matmul(
                                        pss[:, QS_W:2 * QS_W],
                                        krT[64:128, kbl], qrT[g][64:128, qsl],
                                        start=True, stop=True)
                                    pt = ptp.tile([128, 2 * QS_W], F32R, tag="pt")
                                    nc.scalar.activation(
                                        out=pt[:], in_=pss[:],
                                        func=mybir.ActivationFunctionType.Exp,
                                        scale=float(HD) ** -0.5)
                                    if dlt >= 0:
                                        eng = nc.vector if ((kb + g) % 2 == 0) else nc.gpsimd
                                        eng.tensor_tensor(out=pt[:], in0=pt[:],
                                                          in1=mask_sb[:, dlt, 0:2 * QS_W],
                                                          op=mybir.AluOpType.mult)
                                    for u in range(2):
                                        h = 2 * g + u
                                        nc.tensor.matmul(pav[:, h * QS_W:(h + 1) * QS_W],
                                                         v_aug[:, b * NKB + kb, :],
                                                         pt[:, u * QS_W:(u + 1) * QS_W],
                                                         start=(kb == 0), stop=(kb == n_kb - 1))
                            # copy accumulator out of PSUM at once (frees the
                            # bank for the next span; the slow normalize chain
                            # below then runs off the PE critical path)
                            pavs = normp.tile([65, HPC * QS_W], F32, tag="pavs")
                            nc.vector.tensor_copy(out=pavs[:], in_=pav[:])
                            den = normp.tile([1, HPC * QS_W], F32, tag="den")
                            nc.vector.reciprocal(out=den[:], in_=pavs[64:65, :])
                            rb = normp.tile([64, HPC * QS_W], F32, tag="rb")
                            nc.gpsimd.partition_broadcast(rb[:], den[:])
                            for h in range(HPC):
                                nc.vector.tensor_tensor(
                                    out=outT[h // 2][(h % 2) * 64:(h % 2 + 1) * 64, qsl],
                                    in0=pavs[0:64, h * QS_W:(h + 1) * QS_W],
                                    in1=rb[:, h * QS_W:(h + 1) * QS_W],
                                    op=mybir.AluOpType.mult)

                # ---- stage 3: out-projection, then one ReduceScatter ----
                with (
                    tc.tile_pool(name="ystg", bufs=4) as ystg,
                    tc.tile_pool(name="finp", bufs=2) as finp,
                    tc.tile_pool(name="ps_y", bufs=4, space="PSUM") as ps_y,
                ):
                    for dc in range(DCB):
                        for q2 in range(RS_N):
                            q2l = slice(q2 * RS_W, (q2 + 1) * RS_W)
                            py = ps_y.tile([128, RS_W], F32, tag="py")
                            nc.tensor.matmul(py[:],
                                             wo_sb[0][:, dc * 128:(dc + 1) * 128],
                                             outT[0][:, q2l], start=True, stop=False)
                            nc.tensor.matmul(py[:],
                                             wo_sb[1][:, dc * 128:(dc + 1) * 128],
                                             outT[1][:, q2l], start=False, stop=True)
                            ys = ystg.tile([128, RS_W], F32, tag="ys")
                            if (dc + q2) % 2 == 0:
                                nc.vector.tensor_copy(out=ys[:], in_=py[:])
                            else:
                                nc.scalar.copy(out=ys[:], in_=py[:])
                            DMA.dma_start(out=yT_part[dc * 128:(dc + 1) * 128, q2l],
                                          in_=ys[:])
                    nc.gpsimd.collective_compute(
                        "ReduceScatter", mybir.AluOpType.add,
                        replica_groups=[list(range(NC))],
                        ins=[yT_part[:]], outs=[rs_out[:]],
                    )
                    for t in range(2):
                        ft = finp.tile([128, R], F32, tag="ft")
                        DMA.dma_start(out=ft[:], in_=rs_out[t * 128:(t + 1) * 128, :])
                        nc.scalar.activation(out=ft[:], in_=ft[:],
                                             func=mybir.ActivationFunctionType.Identity,
                                             bias=bo_sb[:, t:t + 1], scale=1.0)
                        DMA.dma_start(out=y_sh[t * 128:(t + 1) * 128, :], in_=ft[:])

    nc.finalize()
    return nc


def _rope_perm():
    return np.concatenate([np.arange(0, HD, 2), np.arange(1, HD, 2)])


def _host_prep(x, Wq, Wk, Wv, bv, Wo, bo):
    """Build per-core input maps (inputs pre-tiled to SBUF layouts)."""
    perm = _rope_perm()

    # x tiled: A[kb, p, r] = x[r, kb*128+p];  xta = kb 0..7, xtb = kb 8..15
    A = np.ascontiguousarray(x.reshape(R, D).T).reshape(D // 128, 128, R)
    xta = np.ascontiguousarray(
        A[0:8].reshape(8, 128, RS_N, RS_W).transpose(2, 1, 0, 3)).astype(np.float32)
    xtb = np.ascontiguousarray(
        A[8:16].reshape(8, 128, RS_N, RS_W).transpose(2, 1, 0, 3)).astype(np.float32)

    theta = (1.0 / ROPE_BASE ** (np.arange(0, HD, 2, dtype=np.float64) / HD))
    freqs = np.arange(S, dtype=np.float64)[None, :] * theta[:, None]   # [32, S]
    c4h = np.tile(np.cos(freqs).astype(np.float32), (4, 1))
    s4h = np.tile(np.sin(freqs).astype(np.float32), (4, 1))

    p2 = np.zeros((128, 128), dtype=np.float32)
    for p in list(range(0, 32)) + list(range(64, 96)):
        p2[p + 32, p] = -1.0
    for p in list(range(32, 64)) + list(range(96, 128)):
        p2[p - 32, p] = 1.0

    ident = np.eye(64, dtype=np.float32)
    ones32 = np.ones((128, R // KB_W), dtype=np.float32)

    masks = np.zeros((128, 4, HPC * QS_W), dtype=np.float32)
    for t in range(4):
        m = (np.arange(QS_W)[None, :] >= (t * 128 + np.arange(128))[:, None])
        masks[:, t, :] = np.tile(m.astype(np.float32), (1, HPC))

    in_maps = []
    for c in range(NC):
        wq_c = np.empty((D, 256), dtype=np.float32)
        for cb in range(2):
            for u in range(2):
                h = 4 * c + 2 * cb + u
                wq_c[:, cb * 128 + u * 64: cb * 128 + (u + 1) * 64] = Wq[:, h * 64 + perm]
        wq_t = np.ascontiguousarray(
            wq_c.reshape(D // 128, 128, 256).transpose(1, 0, 2))
        wkv_c = np.empty((D, 128), dtype=np.float32)
        wkv_c[:, 0:64] = Wk[:, c * 64 + perm]
        wkv_c[:, 64:128] = Wv[:, c * 64: (c + 1) * 64]
        wkv_t = np.ascontiguousarray(
            wkv_c.reshape(D // 128, 128, 128).transpose(1, 0, 2))
        wo_c = np.ascontiguousarray(Wo[c * 256:(c + 1) * 256, :]).astype(np.float32)
        bv_c = bv[c * 64:(c + 1) * 64].astype(np.float32).reshape(HD, 1)
        bo_c = np.ascontiguousarray(
            bo[c * 256:(c + 1) * 256].astype(np.float32).reshape(2, 128).T)
        in_maps.append({
            "xta": xta, "xtb": xtb, "wq": wq_t, "wkv": wkv_t, "wo": wo_c,
            "bv": bv_c, "bo": bo_c, "c4h": c4h, "s4h": s4h,
            "p2": p2, "ident": ident, "masks": masks, "ones32": ones32,
        })
    return in_maps


def _run(in_maps, trace=False):
    if "nc" not in _CACHE:
        _CACHE["nc"] = _build()
    try:
        res = run_bass_kernel_spmd(_CACHE["nc"], in_maps,
                                   core_ids=list(range(NC)), trace=trace)
    except Exception:
        # transient device wedge happens occasionally; one retry clears it
        res = run_bass_kernel_spmd(_CACHE["nc"], in_maps,
                                   core_ids=list(range(NC)), trace=trace)
    _CACHE["last_res"] = res
    return res


def _assemble(res):
    yT = np.concatenate([res.results[c]["y_sh"] for c in range(NC)], axis=0)
    return np.ascontiguousarray(yT.T).reshape(B, S, D).astype(np.float32)


def kernel(x, Wq, Wk, Wv, bv, Wo, bo, mask):
    """Full inputs -> full output (B, S, D). `mask` is the causal tril mask
    from setup_inputs; causality is hardcoded so it is not shipped to device."""
    in_maps = _host_prep(np.asarray(x), np.asarray(Wq), np.asarray(Wk),
                         np.asarray(Wv), np.asarray(bv), np.asarray(Wo),
                         np.asarray(bo))
    res = _run(in_maps, trace=False)
    return _assemble(res)


def kernel_timed(x, Wq, Wk, Wv, bv, Wo, bo, mask):
    """Like kernel() but with NTFF tracing; returns (y, exec_time_ns)."""
    in_maps = _host_prep(np.asarray(x), np.asarray(Wq), np.asarray(Wk),
                         np.asarray(Wv), np.asarray(bv), np.asarray(Wo),
                         np.asarray(bo))
    res = _run(in_maps, trace=True)
    return _assemble(res), res.exec_time_ns



# revision 7
# speedup vs baseline: 1.6963x; 1.6963x over previous
"""Trainium2 Bass kernel for causal GQA multi-head attention (nn_MHA_79362405695575).

Full (unsharded) inputs -> full output. Internally: tensor-parallel over heads
across 8 NeuronCores for QKV projection + attention; then an AllToAll moves the
(small, final) attention outputs so each core owns 512 rows, and the output
projection runs row-parallel with no further reduction. Core c returns
yT [2048, 512] fp32 for rows [512c, 512c+512); the host transposes/concats.

Reference semantics (fp32):
  q = x@Wq; k = x@Wk; v = x@Wv + bv           (B=2, S=2048, D=2048)
  q,k := interleaved RoPE(base 10000, hd=64)
  scores = q k^T / 8 (causal), attn = softmax
  out = attn @ v;  y = out @ Wo + bo

Projections and scores run as float32r (TF32-class). The softmax weights,
v, attention outputs, and the whole out-projection run in bf16 (all values
O(1); PSUM accumulation stays fp32). Softmax is max-free and denominators
ride along the AV matmul as a 65th column of v. Only the 128x128 diagonal
tiles are masked (single tril mask); fully-masked regions skip exp/score
work where the free dim stays >=256.
"""

import numpy as np

import concourse.bass as bass
import concourse.tile as tile
from concourse import bacc, mybir
from concourse.bass_utils import run_bass_kernel_spmd

# ---- problem constants (hardcoded; kernel.py must be self-contained) ----
B, S, D = 2, 2048, 2048
NH, NKV, HD = 32, 8, 64
ROPE_BASE = 10000.0
NC = 8                    # cores
HPC = NH // NC            # q heads per core = 4
R = B * S                 # 4096 rows
RS_N = 8                  # projection row spans
RS_W = R // RS_N          # 512 rows per span
QS_W = 512                # attention q-span width
QS_N = S // QS_W          # 4 q spans per batch
KB_W = 128                # k block width
NKB = S // KB_W           # 16 k blocks per batch
EB_N = 16                 # out-proj contraction blocks (2048 head dims / 128)
DC_N = 16                 # out-proj column blocks (2048 / 128)

F32 = mybir.dt.float32
F32R = mybir.dt.float32r
BF16 = mybir.dt.bfloat16

_CACHE = {}


def _build():
    nc = bacc.Bacc("TRN2", target_bir_lowering=False, debug=False, num_devices=NC)

    # ---- DRAM I/O (pre-tiled on host) ----
    xta = nc.dram_tensor("xta", [RS_N, 128, 8, RS_W], F32R, kind="ExternalInput").ap()
    xtb = nc.dram_tensor("xtb", [RS_N, 128, 8, RS_W], F32R, kind="ExternalInput").ap()
    wq = nc.dram_tensor("wq", [128, D // 128, 256], F32R, kind="ExternalInput").ap()
    wkv = nc.dram_tensor("wkv", [128, D // 128, 128], F32R, kind="ExternalInput").ap()
    wo = nc.dram_tensor("wo", [DC_N, 128, EB_N, 128], BF16, kind="ExternalInput").ap()
    bv_in = nc.dram_tensor("bv", [HD, 1], F32, kind="ExternalInput").ap()
    bo_in = nc.dram_tensor("bo", [128, DC_N], F32, kind="ExternalInput").ap()
    c4h = nc.dram_tensor("c4h", [128, S], F32, kind="ExternalInput").ap()
    s4h = nc.dram_tensor("s4h", [128, S], F32, kind="ExternalInput").ap()
    p2 = nc.dram_tensor("p2", [128, 128], F32R, kind="ExternalInput").ap()
    ident = nc.dram_tensor("ident", [64, 64], F32R, kind="ExternalInput").ap()
    tril = nc.dram_tensor("tril", [128, 128], BF16, kind="ExternalInput").ap()
    ones32 = nc.dram_tensor("ones32", [128, R // KB_W], BF16, kind="ExternalInput").ap()
    y_sh = nc.dram_tensor("y_sh", [D, RS_W], F32, kind="ExternalOutput").ap()

    DMA = nc.sync

    with tile.TileContext(nc) as tc:
        with (
            tc.tile_pool(name="persist", bufs=1) as pp,
            tc.tile_pool(name="dram", bufs=1, space="DRAM") as dram,
        ):
            # ---- persistent SBUF (whole kernel) ----
            qrT = [pp.tile([128, R], F32R, tag=f"qrT{t}", name=f"qrT{t}") for t in range(2)]
            krT = pp.tile([128, R], F32R, tag="krT")
            v_aug = pp.tile([128, R // KB_W, 65], BF16, tag="vaug")
            outT = [pp.tile([128, R], BF16, tag=f"outT{t}", name=f"outT{t}") for t in range(2)]
            p2_sb = pp.tile([128, 128], F32R, tag="p2")
            id_sb = pp.tile([64, 64], F32R, tag="ident")
            tril_sb = pp.tile([128, 128], BF16, tag="tril")
            bv_sb = pp.tile([HD, 1], F32, tag="bv")
            bo_sb = pp.tile([128, DC_N], F32, tag="bo")

            DMA.dma_start(out=p2_sb[:], in_=p2[:])
            DMA.dma_start(out=id_sb[:], in_=ident[:])
            DMA.dma_start(out=tril_sb[:], in_=tril[:])
            DMA.dma_start(out=bv_sb[:], in_=bv_in[:])
            DMA.dma_start(out=bo_sb[:], in_=bo_in[:])
            DMA.dma_start(out=v_aug[:, :, 64:65],
                          in_=ones32.rearrange("p (j o) -> p j o", o=1))

            a2a_in = dram.tile([NC, 2 * 128, QS_W], BF16)
            a2a_out = dram.tile([NC, 2 * 128, QS_W], BF16)

            # ================= stage 1: projections + RoPE =================
            with (
                tc.tile_pool(name="w1p", bufs=1) as w1p,
                tc.tile_pool(name="xtpa", bufs=2) as xtpa,
                tc.tile_pool(name="xtpb", bufs=2) as xtpb,
                tc.tile_pool(name="ropet", bufs=2) as ropet,
                tc.tile_pool(name="vstg", bufs=2) as vstg,
                tc.tile_pool(name="ps_q", bufs=2, space="PSUM") as ps_q,
                tc.tile_pool(name="ps_kv", bufs=2, space="PSUM") as ps_kv,
                tc.tile_pool(name="ps_sw", bufs=2, space="PSUM") as ps_sw,
                tc.tile_pool(name="ps_vt", bufs=1, space="PSUM") as ps_vt,
            ):
                wq_sb = w1p.tile([128, D // 128, 256], F32R, tag="wq")
                wkv_sb = w1p.tile([128, D // 128, 128], F32R, tag="wkv")
                c4_sb = w1p.tile([128, S], F32, tag="c4")
                s4_sb = w1p.tile([128, S], F32, tag="s4")
                DMA.dma_start(out=wq_sb[:], in_=wq[:])
                DMA.dma_start(out=wkv_sb[:], in_=wkv[:])
                DMA.dma_start(out=c4_sb[:], in_=c4h[:])
                DMA.dma_start(out=s4_sb[:], in_=s4h[:])
                SPB = RS_N // B          # spans per batch
                for rs in range(RS_N):
                    rsl = slice(rs * RS_W, (rs + 1) * RS_W)
                    ssl = slice((rs % SPB) * RS_W, (rs % SPB + 1) * RS_W)
                    xa = xtpa.tile([128, 8, RS_W], F32R, tag="xa")
                    xb = xtpb.tile([128, 8, RS_W], F32R, tag="xb")
                    DMA.dma_start(out=xa[:], in_=xta[rs])
                    DMA.dma_start(out=xb[:], in_=xtb[rs])

                    def xt(kb):
                        return xa[:, kb, :] if kb < 8 else xb[:, kb - 8, :]

                    # -- q projection: 2 colblocks (2 heads each) --
                    for cb in range(2):
                        pq = ps_q.tile([128, RS_W], F32, tag="pq")
                        for kb in range(D // 128):
                            nc.tensor.matmul(pq[:], wq_sb[:, kb, cb * 128:(cb + 1) * 128],
                                             xt(kb),
                                             start=(kb == 0), stop=(kb == D // 128 - 1))
                        # RoPE: qr = pq*C + P2.T @ (pq*S)
                        st = ropet.tile([128, RS_W], F32R, tag="st")
                        nc.vector.tensor_tensor(out=st[:], in0=pq[:], in1=s4_sb[:, ssl],
                                                op=mybir.AluOpType.mult)
                        sw = ps_sw.tile([128, RS_W], F32, tag="sw")
                        nc.tensor.matmul(sw[:], p2_sb[:], st[:], start=True, stop=True)
                        ct = ropet.tile([128, RS_W], F32, tag="ct")
                        nc.vector.tensor_tensor(out=ct[:], in0=pq[:], in1=c4_sb[:, ssl],
                                                op=mybir.AluOpType.mult)
                        nc.vector.tensor_tensor(out=qrT[cb][:, rsl], in0=ct[:], in1=sw[:],
                                                op=mybir.AluOpType.add)

                    # -- kv projection: cols 0:64 = kT(perm), 64:128 = vT --
                    pkv = ps_kv.tile([128, RS_W], F32, tag="pkv")
                    for kb in range(D // 128):
                        nc.tensor.matmul(pkv[:], wkv_sb[:, kb, :], xt(kb),
                                         start=(kb == 0), stop=(kb == D // 128 - 1))
                    # k RoPE (partitions 0:64), duplicated into krT[0:64] and [64:128]
                    stk = ropet.tile([64, RS_W], F32R, tag="stk")
                    nc.vector.tensor_tensor(out=stk[:], in0=pkv[0:64, :],
                                            in1=s4_sb[0:64, ssl], op=mybir.AluOpType.mult)
                    swk = ps_sw.tile([64, RS_W], F32, tag="sw")
                    nc.tensor.matmul(swk[:], p2_sb[0:64, 0:64], stk[:], start=True, stop=True)
                    ctk = ropet.tile([64, RS_W], F32, tag="ctk")
                    nc.vector.tensor_tensor(out=ctk[:], in0=pkv[0:64, :],
                                            in1=c4_sb[0:64, ssl], op=mybir.AluOpType.mult)
                    nc.vector.tensor_tensor(out=krT[0:64, rsl], in0=ctk[:], in1=swk[:],
                                            op=mybir.AluOpType.add)
                    nc.vector.tensor_tensor(out=krT[64:128, rsl], in0=ctk[:], in1=swk[:],
                                            op=mybir.AluOpType.add)

                    # v: bias add then transpose [64,128] -> [128,64] blocks
                    vst = vstg.tile([64, RS_W], F32R, tag="vst")
                    nc.scalar.activation(out=vst[:], in_=pkv[64:128, :],
                                         func=mybir.ActivationFunctionType.Identity,
                                         bias=bv_sb[:], scale=1.0)
                    for j in range(RS_W // KB_W):
                        pv = ps_vt.tile([128, 64], F32R, tag="pv")
                        nc.tensor.transpose(pv[:], vst[:, j * 128:(j + 1) * 128], id_sb[:])
                        nc.vector.tensor_copy(
                            out=v_aug[:, rs * (RS_W // KB_W) + j, 0:64], in_=pv[:])

            # ================= stage 2: attention =================
            with (
                tc.tile_pool(name="ptp", bufs=3) as ptp,
                tc.tile_pool(name="normp", bufs=2) as normp,
                tc.tile_pool(name="ps_s", bufs=2, space="PSUM") as ps_s,
                tc.tile_pool(name="ps_av", bufs=1, space="PSUM") as ps_av,
            ):
                for b in range(B):
                    for qs in range(QS_N):
                        span = b * QS_N + qs
                        n_kb = 4 * (qs + 1)
                        qsl = slice(b * S + qs * QS_W, b * S + (qs + 1) * QS_W)
                        pav = ps_av.tile([65, HPC * QS_W], F32, tag="pav")
                        for kb in range(n_kb):
                            kbl = slice(b * S + kb * KB_W, b * S + (kb + 1) * KB_W)
                            dlt = kb - 4 * qs
                            # fully-masked q columns of this k block: q < 128*dlt
                            q0 = 128 * dlt if 0 < dlt <= 2 else 0   # score narrowing
                            e0 = 128 * dlt if dlt > 0 else 0        # exp narrowing
                            qn = slice(b * S + qs * QS_W + q0, b * S + (qs + 1) * QS_W)
                            for g in range(2):
                                pss = ps_s.tile([128, 2 * QS_W], F32, tag="pss")
                                nc.tensor.matmul(
                                    pss[:, q0:QS_W],
                                    krT[0:64, kbl],
                                    qrT[g][0:64, qn],
                                    start=True, stop=True)
                                nc.tensor.matmul(
                                    pss[:, QS_W + q0:2 * QS_W],
                                    krT[64:128, kbl],
                                    qrT[g][64:128, qn],
                                    start=True, stop=True)
                                pt = ptp.tile([128, 2 * QS_W], BF16, tag="pt")
                                nc.scalar.activation(
                                    out=pt[:, e0:2 * QS_W], in_=pss[:, e0:2 * QS_W],
                                    func=mybir.ActivationFunctionType.Exp,
                                    scale=float(HD) ** -0.5)
                                if dlt >= 0:
                                    if e0 > 0:
                                        # zero the fully-masked leading cols of
                                        # each head (stale / garbage data there)
                                        nc.vector.memset(pt[:, 0:e0], 0.0)
                                        nc.vector.memset(pt[:, QS_W:QS_W + e0], 0.0)
                                    # triangular mask on the 128-wide diagonal window
                                    for u in range(2):
                                        w0 = u * QS_W + 128 * dlt
                                        nc.vector.tensor_tensor(
                                            out=pt[:, w0:w0 + 128],
                                            in0=pt[:, w0:w0 + 128],
                                            in1=tril_sb[:],
                                            op=mybir.AluOpType.mult)
                                for u in range(2):
                                    h = 2 * g + u
                                    nc.tensor.matmul(pav[:, h * QS_W:(h + 1) * QS_W],
                                                     v_aug[:, b * NKB + kb, :],
                                                     pt[:, u * QS_W:(u + 1) * QS_W],
                                                     start=(kb == 0), stop=(kb == n_kb - 1))
                        # copy accumulator out of PSUM at once (frees the bank
                        # for the next span); normalization then runs off the
                        # PE critical path
                        pavs = normp.tile([65, HPC * QS_W], F32, tag="pavs")
                        nc.vector.tensor_copy(out=pavs[:], in_=pav[:])
                        # 1/den via exp(-ln(den)) on the scalar engine: both
                        # functions live in one activation-table set, so this
                        # stays off the DVE critical path with no table swaps
                        dln = normp.tile([1, HPC * QS_W], F32, tag="dln")
                        nc.scalar.activation(out=dln[:], in_=pavs[64:65, :],
                                             func=mybir.ActivationFunctionType.Ln)
                        rden = normp.tile([1, HPC * QS_W], F32, tag="rden")
                        nc.scalar.activation(out=rden[:], in_=dln[:],
                                             func=mybir.ActivationFunctionType.Exp,
                                             scale=-1.0)
                        rb = normp.tile([64, HPC * QS_W], F32, tag="rb")
                        nc.gpsimd.partition_broadcast(rb[:], rden[:])
                        for h in range(HPC):
                            nc.vector.tensor_tensor(
                                out=outT[h // 2][(h % 2) * 64:(h % 2 + 1) * 64, qsl],
                                in0=pavs[0:64, h * QS_W:(h + 1) * QS_W],
                                in1=rb[:, h * QS_W:(h + 1) * QS_W],
                                op=mybir.AluOpType.mult)
                        # ship this span's slice to the all-to-all input buffer
                        for t in range(2):
                            DMA.dma_start(out=a2a_in[span, t * 128:(t + 1) * 128, :],
                                          in_=outT[t][:, qsl])

            # ====== stage 3: all-to-all, then row-parallel out-projection ======
            nc.gpsimd.collective_compute(
                "AllToAll", mybir.AluOpType.bypass,
                replica_groups=[list(range(NC))],
                ins=[a2a_in[:]], outs=[a2a_out[:]],
            )
            with (
                tc.tile_pool(name="ofp", bufs=1) as ofp,
                tc.tile_pool(name="wop", bufs=2) as wop,
                tc.tile_pool(name="ystg", bufs=4) as ystg,
                tc.tile_pool(name="ps_y", bufs=4, space="PSUM") as ps_y,
            ):
                of = ofp.tile([128, EB_N, QS_W], BF16, tag="of")
                for sc in range(NC):
                    for t in range(2):
                        DMA.dma_start(out=of[:, sc * 2 + t, :],
                                      in_=a2a_out[sc, t * 128:(t + 1) * 128, :])
                for dc in range(DC_N):
                    wo_sb = wop.tile([128, EB_N, 128], BF16, tag="wo")
                    DMA.dma_start(out=wo_sb[:], in_=wo[dc])
                    py = ps_y.tile([128, QS_W], F32, tag="py")
                    for eb in range(EB_N):
                        nc.tensor.matmul(py[:], wo_sb[:, eb, :], of[:, eb, :],
                                         start=(eb == 0), stop=(eb == EB_N - 1))
                    ys = ystg.tile([128, QS_W], F32, tag="ys")
                    nc.scalar.activation(out=ys[:], in_=py[:],
                                         func=mybir.ActivationFunctionType.Identity,
                                         bias=bo_sb[:, dc:dc + 1], scale=1.0)
                    DMA.dma_start(out=y_sh[dc * 128:(dc + 1) * 128, :], in_=ys[:])

    nc.finalize()
    return nc


def _rope_perm():
    return np.concatenate([np.arange(0, HD, 2), np.arange(1, HD, 2)])


def _to_bf16(a):
    import ml_dtypes
    return np.asarray(a, dtype=ml_dtypes.bfloat16)


def _host_prep(x, Wq, Wk, Wv, bv, Wo, bo):
    """Build per-core input maps (inputs pre-tiled to SBUF layouts)."""
    perm = _rope_perm()

    # x tiled: A[kb, p, r] = x[r, kb*128+p];  xta = kb 0..7, xtb = kb 8..15
    A = np.ascontiguousarray(x.reshape(R, D).T).reshape(D // 128, 128, R)
    xta = np.ascontiguousarray(
        A[0:8].reshape(8, 128, RS_N, RS_W).transpose(2, 1, 0, 3)).astype(np.float32)
    xtb = np.ascontiguousarray(
        A[8:16].reshape(8, 128, RS_N, RS_W).transpose(2, 1, 0, 3)).astype(np.float32)

    theta = (1.0 / ROPE_BASE ** (np.arange(0, HD, 2, dtype=np.float64) / HD))
    freqs = np.arange(S, dtype=np.float64)[None, :] * theta[:, None]   # [32, S]
    c4h = np.tile(np.cos(freqs).astype(np.float32), (4, 1))
    s4h = np.tile(np.sin(freqs).astype(np.float32), (4, 1))

    p2 = np.zeros((128, 128), dtype=np.float32)
    for p in list(range(0, 32)) + list(range(64, 96)):
        p2[p + 32, p] = -1.0
    for p in list(range(32, 64)) + list(range(96, 128)):
        p2[p - 32, p] = 1.0

    ident = np.eye(64, dtype=np.float32)
    ones32 = _to_bf16(np.ones((128, R // KB_W), dtype=np.float32))
    tril = _to_bf16((np.arange(128)[None, :] >= np.arange(128)[:, None])
                    .astype(np.float32))

    # Wo reordered to the on-device head-dim order (identical for all cores):
    # contraction block eb=(s,t): partitions p -> head 4s+2t+p//64, dim p%64
    woh = np.empty((DC_N, 128, EB_N, 128), dtype=np.float32)
    Wor = Wo.reshape(NH, HD, D)
    for s in range(NC):
        for t in range(2):
            eb = 2 * s + t
            rows = np.concatenate([Wor[4 * s + 2 * t], Wor[4 * s + 2 * t + 1]],
                                  axis=0)  # [128, D]
            # rows.T.reshape -> [dc, j, p]; need [dc, p, j]
            woh[:, :, eb, :] = rows.T.reshape(DC_N, 128, 128).transpose(0, 2, 1)
    woh = _to_bf16(woh)

    boh = np.ascontiguousarray(
        bo.astype(np.float32).reshape(DC_N, 128).T)  # [128, DC_N]

    in_maps = []
    for c in range(NC):
        wq_c = np.empty((D, 256), dtype=np.float32)
        for cb in range(2):
            for u in range(2):
                h = 4 * c + 2 * cb + u
                wq_c[:, cb * 128 + u * 64: cb * 128 + (u + 1) * 64] = Wq[:, h * 64 + perm]
        wq_t = np.ascontiguousarray(
            wq_c.reshape(D // 128, 128, 256).transpose(1, 0, 2))
        wkv_c = np.empty((D, 128), dtype=np.float32)
        wkv_c[:, 0:64] = Wk[:, c * 64 + perm]
        wkv_c[:, 64:128] = Wv[:, c * 64: (c + 1) * 64]
        wkv_t = np.ascontiguousarray(
            wkv_c.reshape(D // 128, 128, 128).transpose(1, 0, 2))
        bv_c = bv[c * 64:(c + 1) * 64].astype(np.float32).reshape(HD, 1)
        in_maps.append({
            "xta": xta, "xtb": xtb, "wq": wq_t, "wkv": wkv_t, "wo": woh,
            "bv": bv_c, "bo": boh, "c4h": c4h, "s4h": s4h,
            "p2": p2, "ident": ident, "tril": tril, "ones32": ones32,
        })
    return in_maps


def _run(in_maps, trace=False):
    if "nc" not in _CACHE:
        _CACHE["nc"] = _build()
    try:
        res = run_bass_kernel_spmd(_CACHE["nc"], in_maps,
                                   core_ids=list(range(NC)), trace=trace)
    except Exception:
        # transient device wedge happens occasionally; one retry clears it
        res = run_bass_kernel_spmd(_CACHE["nc"], in_maps,
                                   core_ids=list(range(NC)), trace=trace)
    _CACHE["last_res"] = res
    return res


def _assemble(res):
    y = np.empty((R, D), dtype=np.float32)
    for c in range(NC):
        y[c * RS_W:(c + 1) * RS_W, :] = res.results[c]["y_sh"].T
    return y.reshape(B, S, D)


def kernel(x, Wq, Wk, Wv, bv, Wo, bo, mask):
    """Full inputs -> full output (B, S, D). `mask` is the causal tril mask
    from setup_inputs; causality is hardcoded so it is not shipped to device."""
    in_maps = _host_prep(np.asarray(x), np.asarray(Wq), np.asarray(Wk),
                         np.asarray(Wv), np.asarray(bv), np.asarray(Wo),
                         np.asarray(bo))
    res = _run(in_maps, trace=False)
    return _assemble(res)


def kernel_timed(x, Wq, Wk, Wv, bv, Wo, bo, mask):
    """Like kernel() but with NTFF tracing; returns (y, exec_time_ns)."""
    in_maps = _host_prep(np.asarray(x), np.asarray(Wq), np.asarray(Wk),
                         np.asarray(Wv), np.asarray(bv), np.asarray(Wo),
                         np.asarray(bo))
    res = _run(in_maps, trace=True)
    return _assemble(res), res.exec_time_ns
